# Initial kernel scaffold
#
"""Trainium2 Bass kernel for BuiltSWAP: out = (state_re + i*state_im) @ M.

M is a real [8192, 8192] matrix (a 0/1 SWAP-gate permutation in practice, but
treated as a dense matrix).  Since M is real the complex matmul decomposes into
two real matmuls sharing the same rhs:

    out_re = state_re @ M          out_im = state_im @ M

Strategy (8 NeuronCores, column-sharded tensor parallelism):
  - Shard M's columns across the 8 cores (1024 cols each): each core computes
    state[64, 8192] @ M_shard[8192, 1024] for re and im, outputs concatenated.
  - Fuse re and im into a single 128-row lhsT (64 re rows + 64 im rows) so the
    128x128 PE array is fully used.
  - Precision: split fp32 state into bf16 hi + bf16 lo (state = hi + lo), run
    two bf16 matmul passes accumulating into the same fp32 PSUM.  M's entries
    are exactly representable in bf16, so the result carries ~17 mantissa bits
    of the state -- rel err ~1e-5 vs fp32.  This halves M's HBM traffic and
    doubles PE throughput vs fp32 matmuls.
"""

import numpy as np
import ml_dtypes

BATCH = 64
N = 8192
NCORES = 8
COLS = N // NCORES          # 1024 output columns per core
P = 128                     # partitions
KT = N // P                 # 64 k-tiles
NCH = COLS // 512           # 2 psum chunks of 512
KBLK = 8                    # k-tiles per DMA block
NBLK = KT // KBLK           # 8 DMA blocks

bf16 = ml_dtypes.bfloat16

_cached = {}


def _build_program():
    import concourse.bass as bass
    import concourse.mybir as mybir
    import concourse.tile as tile

    nc = bass.Bass()
    st_d = nc.declare_dram_parameter("st", [P, KT, 256], mybir.dt.bfloat16, isOutput=False)
    m_d = nc.declare_dram_parameter("m", [P, KT, NCH, 512], mybir.dt.bfloat16, isOutput=False)
    out_d = nc.declare_dram_parameter("out", [P, COLS], mybir.dt.float32, isOutput=True)

    with tile.TileContext(nc) as tc:
        with (
            tc.tile_pool(name="stp", bufs=1) as stp,
            tc.tile_pool(name="mp", bufs=3) as mp,
            tc.tile_pool(name="op", bufs=1) as op,
            tc.tile_pool(name="ps", bufs=1, space="PSUM") as ps,
        ):
            st_sb = stp.tile([P, KT, 256], mybir.dt.bfloat16)
            # split the state load so the first matmuls aren't gated on 4MB
            for kb in range(NBLK):
                nc.sync.dma_start(
                    st_sb[:, kb * KBLK:(kb + 1) * KBLK, :],
                    st_d[:, kb * KBLK:(kb + 1) * KBLK, :],
                )
            out_sb = op.tile([P, COLS], mybir.dt.float32)
            psums = [ps.tile([P, 512], mybir.dt.float32) for _ in range(NCH)]

            for kb in range(NBLK):
                m_sb = mp.tile([P, KBLK, NCH, 512], mybir.dt.bfloat16)
                nc.sync.dma_start(m_sb[:], m_d[:, kb * KBLK:(kb + 1) * KBLK, :, :])
                for kj in range(KBLK):
                    ko = kb * KBLK + kj
                    for nch in range(NCH):
                        nc.tensor.matmul(
                            psums[nch][:],
                            st_sb[:, ko, 0:128],
                            m_sb[:, kj, nch, :],
                            start=(ko == 0),
                            stop=False,
                        )
                        nc.tensor.matmul(
                            psums[nch][:],
                            st_sb[:, ko, 128:256],
                            m_sb[:, kj, nch, :],
                            start=False,
                            stop=(ko == KT - 1),
                        )
            for nch in range(NCH):
                nc.vector.tensor_copy(out_sb[:, nch * 512:(nch + 1) * 512], psums[nch][:])
            nc.sync.dma_start(out_d[:], out_sb[:])
    return nc


def _get_program():
    if "nc" not in _cached:
        _cached["nc"] = _build_program()
    return _cached["nc"]


def _prep_inputs(state_re, state_im, M):
    # lhsT layout: [8192, 256] where cols 0:64 re_hi, 64:128 im_hi,
    # 128:192 re_lo, 192:256 im_lo; tiled to [128 part, 64 ktile, 256].
    S = np.empty((N, P), dtype=np.float32)
    S[:, :BATCH] = state_re.T
    S[:, BATCH:] = state_im.T
    hi = S.astype(bf16)
    lo = (S - hi.astype(np.float32)).astype(bf16)
    stall = np.concatenate([hi, lo], axis=1)  # [8192, 256] bf16
    st_tiled = np.ascontiguousarray(
        stall.reshape(KT, P, 256).transpose(1, 0, 2)
    )  # [128, 64, 256]

    Mb = M.astype(bf16)  # [8192, 8192]
    m_tiles = []
    for c in range(NCORES):
        shard = Mb[:, c * COLS:(c + 1) * COLS]
        m_tiles.append(
            np.ascontiguousarray(
                shard.reshape(KT, P, NCH, 512).transpose(1, 0, 2, 3)
            )
        )  # [128, 64, 2, 512]
    return st_tiled, m_tiles


def run_on_hw(state_re, state_im, M, trace=False):
    from concourse.bass_utils import run_bass_kernel_spmd

    nc = _get_program()
    st_tiled, m_tiles = _prep_inputs(state_re, state_im, M)
    in_maps = [{"st": st_tiled, "m": m_tiles[c]} for c in range(NCORES)]
    res = run_bass_kernel_spmd(
        nc, in_maps, list(range(NCORES)), trace=trace,
        trace_cores=list(range(NCORES)) if trace else None,
    )
    full = np.concatenate([res.results[c]["out"] for c in range(NCORES)], axis=1)
    out = (full[:BATCH] + 1j * full[BATCH:]).astype(np.complex64)
    return out, res


def kernel(state_re, state_im, M):
    out, _ = run_on_hw(state_re, state_im, M, trace=False)
    return out


# revision 6
# speedup vs baseline: 1.0760x; 1.0760x over previous
"""Trainium2 Bass kernel for BuiltSWAP: out = (state_re + i*state_im) @ M.

M is a real [8192, 8192] matrix (a 0/1 SWAP-gate permutation in practice, but
treated as a dense matrix).  Since M is real the complex matmul decomposes into
two real matmuls sharing the same rhs:

    out_re = state_re @ M          out_im = state_im @ M

Strategy (8 NeuronCores, column-sharded tensor parallelism):
  - Shard M's columns across the 8 cores (1024 cols each): each core computes
    state[64, 8192] @ M_shard[8192, 1024] for re and im, outputs concatenated.
  - Fuse re and im into a single 128-row lhsT (64 re rows + 64 im rows) so the
    128x128 PE array is fully used.
  - Precision: split fp32 state into bf16 hi + bf16 lo (state = hi + lo), run
    two bf16 matmul passes accumulating into the same fp32 PSUM.  M's entries
    are exactly representable in bf16, so the result carries ~17 mantissa bits
    of the state -- rel err ~1e-5 vs fp32.  This halves M's HBM traffic and
    doubles PE throughput vs fp32 matmuls.
"""

import numpy as np
import ml_dtypes

BATCH = 64
N = 8192
NCORES = 8
COLS = N // NCORES          # 1024 output columns per core
P = 128                     # partitions
KT = N // P                 # 64 k-tiles
NCH = COLS // 512           # 2 psum chunks of 512
KBLK = 8                    # k-tiles per DMA block
NBLK = KT // KBLK           # 8 DMA blocks

bf16 = ml_dtypes.bfloat16

_cached = {}


def _build_program(reps=1):
    import concourse.mybir as mybir
    import concourse.tile as tile
    from concourse import bacc

    nc = bacc.Bacc("TRN2", target_bir_lowering=False, debug=False)
    st_d = nc.declare_dram_parameter("st", [P, KT, 256], mybir.dt.bfloat16, isOutput=False)
    m_d = nc.declare_dram_parameter("m", [P, KT, NCH, 512], mybir.dt.bfloat16, isOutput=False)
    out_d = nc.declare_dram_parameter("out", [P, COLS], mybir.dt.float32, isOutput=True)

    with tile.TileContext(nc) as tc:
        with (
            tc.tile_pool(name="stp", bufs=1) as stp,
            tc.tile_pool(name="mp", bufs=3) as mp,
            tc.tile_pool(name="op", bufs=1) as op,
            tc.tile_pool(name="ps", bufs=1, space="PSUM") as ps,
        ):
            st_sb = stp.tile([P, KT, 256], mybir.dt.bfloat16)
            # split the state load so the first matmuls aren't gated on 4MB
            for kb in range(NBLK):
                nc.sync.dma_start(
                    st_sb[:, kb * KBLK:(kb + 1) * KBLK, :],
                    st_d[:, kb * KBLK:(kb + 1) * KBLK, :],
                )
            for _rep in range(reps):
                out_sb = op.tile([P, COLS], mybir.dt.float32, name="out_sb")
                psums = [
                    ps.tile([P, 512], mybir.dt.float32, name=f"psum{i}")
                    for i in range(NCH)
                ]

                for kb in range(NBLK):
                    m_sb = mp.tile([P, KBLK, NCH, 512], mybir.dt.bfloat16, name="m_sb")
                    nc.sync.dma_start(m_sb[:], m_d[:, kb * KBLK:(kb + 1) * KBLK, :, :])
                    for kj in range(KBLK):
                        ko = kb * KBLK + kj
                        for nch in range(NCH):
                            nc.tensor.matmul(
                                psums[nch][:],
                                st_sb[:, ko, 0:128],
                                m_sb[:, kj, nch, :],
                                start=(ko == 0),
                                stop=False,
                            )
                            nc.tensor.matmul(
                                psums[nch][:],
                                st_sb[:, ko, 128:256],
                                m_sb[:, kj, nch, :],
                                start=False,
                                stop=(ko == KT - 1),
                            )
                for nch in range(NCH):
                    nc.vector.tensor_copy(
                        out_sb[:, nch * 512:(nch + 1) * 512], psums[nch][:]
                    )
                nc.sync.dma_start(out_d[:], out_sb[:])
    nc.compile()
    return nc


def _get_program():
    if "nc" not in _cached:
        _cached["nc"] = _build_program()
    return _cached["nc"]


def _prep_inputs(state_re, state_im, M):
    # lhsT layout: [8192, 256] where cols 0:64 re_hi, 64:128 im_hi,
    # 128:192 re_lo, 192:256 im_lo; tiled to [128 part, 64 ktile, 256].
    S = np.empty((N, P), dtype=np.float32)
    S[:, :BATCH] = state_re.T
    S[:, BATCH:] = state_im.T
    hi = S.astype(bf16)
    lo = (S - hi.astype(np.float32)).astype(bf16)
    stall = np.concatenate([hi, lo], axis=1)  # [8192, 256] bf16
    st_tiled = np.ascontiguousarray(
        stall.reshape(KT, P, 256).transpose(1, 0, 2)
    )  # [128, 64, 256]

    Mb = M.astype(bf16)  # [8192, 8192]
    m_tiles = []
    for c in range(NCORES):
        shard = Mb[:, c * COLS:(c + 1) * COLS]
        m_tiles.append(
            np.ascontiguousarray(
                shard.reshape(KT, P, NCH, 512).transpose(1, 0, 2, 3)
            )
        )  # [128, 64, 2, 512]
    return st_tiled, m_tiles


def run_on_hw(state_re, state_im, M, trace=False):
    from concourse.bass_utils import run_bass_kernel_spmd

    nc = _get_program()
    st_tiled, m_tiles = _prep_inputs(state_re, state_im, M)
    in_maps = [{"st": st_tiled, "m": m_tiles[c]} for c in range(NCORES)]
    res = run_bass_kernel_spmd(
        nc, in_maps, list(range(NCORES)), trace=trace,
        trace_cores=list(range(NCORES)) if trace else None,
    )
    full = np.concatenate([res.results[c]["out"] for c in range(NCORES)], axis=1)
    out = (full[:BATCH] + 1j * full[BATCH:]).astype(np.complex64)
    return out, res


def kernel(state_re, state_im, M):
    out, _ = run_on_hw(state_re, state_im, M, trace=False)
    return out
